# revision 5
# baseline (speedup 1.0000x reference)
"""Distributed 3-layer GAT (PyG GATConv semantics, single head) for TRN2,
running SPMD on 8 NeuronCores via Bass/Tile.

Contract: kernel(**inputs) takes the FULL inputs from setup_inputs()
(z [50000,128] f32, edge_index [2,800000] int32, Ws [3,128,128],
a_src/a_dst [3,128], bias [3,128]) and returns the full [50000,128] f32
output.

Distribution (dst-sharded, per the 1D node-sharding scheme):
  nodes are split into contiguous blocks of NB=ceil(N/8/128)*128 per core;
  every edge lives on the core owning its destination node, so the
  segment-softmax and the scatter-add are core-local. Per layer each core
  computes x_aug = h_own @ [W | W@a_src | W@a_dst] for its own nodes on
  the PE, the x rows + per-node alpha_src are AllGathered into a
  node-major table, and the per-edge phase gathers x[src] rows (512B,
  int16 dma_gather with a low/high table split) plus alpha_d[dst]
  (256B replicated rows), builds A = onehot(dst_local)*exp(lrelu(...))
  with one fused DVE op per 128-edge chunk, and reduces via PE matmuls
  accumulated in PSUM (features x dst-block output + denominator row).
"""
import dataclasses
import numpy as np

import concourse.bass as bass
import concourse.bacc as bacc
import concourse.mybir as mybir
import concourse.tile as tile
from concourse.bass_interp import MultiCoreSim

F32 = mybir.dt.float32
AF = mybir.ActivationFunctionType
ALU = mybir.AluOpType

N_CORES = 8
D = 128
BLK = 128
NEG = 0.2
EPS = 1e-16
N_LAYERS = 3

LAST_EXEC_TIME_NS = None
TRACE = False
DEBUG_TAPS = False
LAST_RESULTS = None
BENCH_ITERS = 0
LAST_BENCH_TIMES = None


@dataclasses.dataclass
class Cfg:
    n_nodes: int
    nb: int
    half: int
    cl: int
    ch: int

    @property
    def nt(self):
        return N_CORES * self.nb

    @property
    def nblk(self):
        return self.nb // BLK

    @property
    def cb(self):
        return self.cl + self.ch


def wrap_idx(flat):
    """dma_gather index layout: idx i read from [i%16, i//16]; the 16-row
    pattern replicated to all 128 partitions."""
    n = len(flat)
    assert n % 16 == 0
    w = np.asarray(flat, np.int16).reshape(n // 16, 16).T
    return np.tile(w, (8, 1))


def prep_host(z, edge_index, Ws, a_src, a_dst, bias, cfg):
    nb, half, cl, ch, cb, nblk = cfg.nb, cfg.half, cfg.cl, cfg.ch, cfg.cb, cfg.nblk
    n = cfg.n_nodes
    loop = np.arange(n, dtype=np.int64)
    src = np.concatenate([edge_index[0].astype(np.int64), loop])
    dst = np.concatenate([edge_index[1].astype(np.int64), loop])

    zT = np.zeros((D, cfg.nt), np.float32)
    zT[:, :n] = np.asarray(z, np.float32).T

    W_aug = np.zeros((D, N_LAYERS, 132), np.float32)
    for l in range(N_LAYERS):
        W_aug[:, l, :D] = Ws[l]
        W_aug[:, l, D] = Ws[l] @ a_src[l]
        W_aug[:, l, D + 1] = Ws[l] @ a_dst[l]
    as_bc = np.zeros((128, N_LAYERS, D), np.float32)
    for l in range(N_LAYERS):
        as_bc[:, l, :] = a_src[l][None, :]
    bias_in = np.zeros((128, N_LAYERS), np.float32)
    bias_in[:, :] = np.asarray(bias, np.float32).T

    in_maps = []
    for r in range(N_CORES):
        m = (dst >= r * nb) & (dst < (r + 1) * nb)
        s, d = src[m], dst[m] - r * nb
        blk = d // BLK
        lh = (s >= half).astype(np.int64)
        order = np.lexsort((s, lh, blk))
        s, d, blk, lh = s[order], d[order], blk[order], lh[order]

        idx_lo = np.zeros((nblk, cl * 128), np.int64)
        idx_hi = np.zeros((nblk, ch * 128), np.int64)
        idx_ad = np.zeros((nblk, cb * 128), np.int64)
        dloc = -np.ones((nblk, cb * 128), np.float32)
        for b in range(nblk):
            bm = blk == b
            slo = s[bm & (lh == 0)]
            shi = s[bm & (lh == 1)]
            dlo = d[bm & (lh == 0)]
            dhi = d[bm & (lh == 1)]
            assert len(slo) <= cl * 128 and len(shi) <= ch * 128
            idx_lo[b, :len(slo)] = slo
            idx_hi[b, :len(shi)] = shi - half
            idx_ad[b, :len(dlo)] = dlo
            idx_ad[b, cl * 128:cl * 128 + len(dhi)] = dhi
            dloc[b, :len(dlo)] = dlo - b * BLK
            dloc[b, cl * 128:cl * 128 + len(dhi)] = dhi - b * BLK

        ilo = np.concatenate([wrap_idx(idx_lo[b]) for b in range(nblk)], axis=1)
        ihi = np.concatenate([wrap_idx(idx_hi[b]) for b in range(nblk)], axis=1)
        iad = np.concatenate([wrap_idx(idx_ad[b]) for b in range(nblk)], axis=1)
        dl = np.concatenate(
            [dloc[b].reshape(cb, 128).T for b in range(nblk)], axis=1
        ).astype(np.float32)

        in_maps.append({
            "zT_own": np.ascontiguousarray(zT[:, r * nb:(r + 1) * nb]),
            "W_aug": W_aug,
            "as_bc": as_bc,
            "bias_in": bias_in,
            "idx_lo_in": np.ascontiguousarray(ilo),
            "idx_hi_in": np.ascontiguousarray(ihi),
            "idx_ad_in": np.ascontiguousarray(iad),
            "dstloc_in": np.ascontiguousarray(dl),
        })
    return in_maps


def pick_cfg(n, edge_index):
    nb = -(-n // (N_CORES * BLK)) * BLK
    half = 32768
    loop = np.arange(n, dtype=np.int64)
    src = np.concatenate([edge_index[0].astype(np.int64), loop])
    dst = np.concatenate([edge_index[1].astype(np.int64), loop])
    mlo = mhi = 0
    for r in range(N_CORES):
        m = (dst >= r * nb) & (dst < (r + 1) * nb)
        s, d = src[m], dst[m] - r * nb
        blk = d // BLK
        for b in range(nb // BLK):
            bm = blk == b
            mlo = max(mlo, int((s[bm] < half).sum()))
            mhi = max(mhi, int((s[bm] >= half).sum()))
    return Cfg(n_nodes=n, nb=nb, half=half,
               cl=-(-mlo // 128), ch=-(-mhi // 128))


def build(nc, tc, cfg):
    nb, nt, nblk, cl, ch, cb = cfg.nb, cfg.nt, cfg.nblk, cfg.cl, cfg.ch, cfg.cb
    half = cfg.half

    zT_own = nc.dram_tensor("zT_own", [D, nb], F32, kind="ExternalInput").ap()
    W_aug = nc.dram_tensor("W_aug", [D, N_LAYERS, 132], F32,
                           kind="ExternalInput").ap()
    as_in = nc.dram_tensor("as_bc", [128, N_LAYERS, D], F32,
                           kind="ExternalInput").ap()
    bias_in = nc.dram_tensor("bias_in", [128, N_LAYERS], F32,
                             kind="ExternalInput").ap()
    idx_lo_in = nc.dram_tensor("idx_lo_in", [128, nblk * cl * 8],
                               mybir.dt.int16, kind="ExternalInput").ap()
    idx_hi_in = nc.dram_tensor("idx_hi_in", [128, nblk * ch * 8],
                               mybir.dt.int16, kind="ExternalInput").ap()
    idx_ad_in = nc.dram_tensor("idx_ad_in", [128, nblk * cb * 8],
                               mybir.dt.int16, kind="ExternalInput").ap()
    dstloc_in = nc.dram_tensor("dstloc_in", [128, nblk * cb], F32,
                               kind="ExternalInput").ap()
    out_own = nc.dram_tensor("out_own", [D, nb], F32, kind="ExternalOutput").ap()
    dbg_h = nc.dram_tensor("dbg_h", [N_LAYERS, D, nb], F32,
                           kind="ExternalOutput").ap() if DEBUG_TAPS else None

    xtab_ins, astab_ins, adreps, xtabs, astabs = [], [], [], [], []
    for l in range(N_LAYERS):
        # per-layer staging tensors: reuse across layers would be a
        # WAR race (layer l+1 writes vs layer l readers) on DRAM.
        xtab_ins.append(nc.dram_tensor(f"xtab_in{l}", [nb, D], F32).ap())
        astab_ins.append(nc.dram_tensor(f"astab_in{l}", [nb], F32).ap())
        adreps.append(nc.dram_tensor(f"adrep{l}", [nb, 64], F32).ap())
        xtabs.append(nc.dram_tensor(f"xtab{l}", [nt, D], F32,
                                    addr_space="Shared").ap())
        astabs.append(nc.dram_tensor(f"astab{l}", [nt], F32,
                                     addr_space="Shared").ap())

    rg = [list(range(N_CORES))]

    with (
        tc.tile_pool(name="const", bufs=1) as cpool,
        tc.tile_pool(name="pers", bufs=1) as pers,
        tc.tile_pool(name="gather", bufs=2) as gpool,
        tc.tile_pool(name="work", bufs=3) as wpool,
        tc.tile_pool(name="blkw", bufs=2) as bpool,
        tc.tile_pool(name="psum", bufs=2, space="PSUM") as ppool,
    ):
        h_own = pers.tile([D, nb], F32)
        hacc = pers.tile([D, nb], F32)
        den_row = pers.tile([1, nb], F32)
        W_sb = cpool.tile([D, N_LAYERS, 132], F32)
        as_sb = cpool.tile([128, N_LAYERS, D], F32)
        bias_sb = cpool.tile([128, N_LAYERS], F32)
        ilo_sb = cpool.tile([128, nblk * cl * 8], mybir.dt.int16)
        ihi_sb = cpool.tile([128, nblk * ch * 8], mybir.dt.int16)
        iad_sb = cpool.tile([128, nblk * cb * 8], mybir.dt.int16)
        dstloc_sb = cpool.tile([128, nblk * cb], F32)
        iota_i = cpool.tile([128, 128], mybir.dt.int32)
        iota_f = cpool.tile([128, 128], F32)
        ones_sb = cpool.tile([128, 1], F32)

        nc.sync.dma_start(h_own[:, :], zT_own)
        nc.sync.dma_start(W_sb[:, :, :], W_aug)
        nc.sync.dma_start(as_sb[:, :, :], as_in)
        nc.sync.dma_start(bias_sb[:, :], bias_in)
        nc.sync.dma_start(ilo_sb[:, :], idx_lo_in)
        nc.sync.dma_start(ihi_sb[:, :], idx_hi_in)
        nc.sync.dma_start(iad_sb[:, :], idx_ad_in)
        nc.sync.dma_start(dstloc_sb[:, :], dstloc_in)
        nc.gpsimd.iota(iota_i[:, :], pattern=[[1, 128]], base=0,
                       channel_multiplier=0)
        nc.vector.tensor_copy(iota_f[:, :], iota_i[:, :])
        nc.vector.memset(ones_sb[:, :], 1.0)

        for l in range(N_LAYERS):
            xtab, astab = xtabs[l], astabs[l]
            xtab_in, astab_in, adrep = xtab_ins[l], astab_ins[l], adreps[l]
            # x phase: own nodes
            for b in range(nblk):
                sl = slice(b * BLK, (b + 1) * BLK)
                psum_x = ppool.tile([128, 132], F32, tag="px")
                nc.tensor.matmul(out=psum_x[:, :], lhsT=h_own[:, sl],
                                 rhs=W_sb[:, l, :], start=True, stop=True)
                xa = wpool.tile([128, 132], F32, tag="xa")
                nc.scalar.copy(xa[:, :], psum_x[:, :])
                ad_rep_t = wpool.tile([128, 64], F32, tag="adr")
                nc.vector.tensor_copy(ad_rep_t[:, :],
                                      xa[:, 129:130].to_broadcast([128, 64]))
                nc.sync.dma_start(xtab_in[sl, :], xa[:, 0:D])
                nc.sync.dma_start(astab_in[sl], xa[:, D:D + 1])
                nc.sync.dma_start(adrep[sl, :], ad_rep_t[:, :])

            nc.gpsimd.collective_compute(
                "AllGather", ALU.bypass, replica_groups=rg,
                ins=[xtab_in[:, :]], outs=[xtab[:, :]])
            nc.gpsimd.collective_compute(
                "AllGather", ALU.bypass, replica_groups=rg,
                ins=[astab_in[:]], outs=[astab[:]])

            # edge phase
            for b in range(nblk):
                sl = slice(b * BLK, (b + 1) * BLK)
                gl = gpool.tile([128, cl, D], F32, tag="gl")
                gh = gpool.tile([128, ch, D], F32, tag="gh")
                gad = gpool.tile([128, cb, 64], F32, tag="gad")
                nc.gpsimd.dma_gather(
                    out_ap=gl[:, :, :], in_ap=xtab[0:half, :],
                    idxs_ap=ilo_sb[:, b * cl * 8:(b + 1) * cl * 8],
                    num_idxs=cl * 128, num_idxs_reg=cl * 128, elem_size=D,
                    single_packet=False)
                nc.gpsimd.dma_gather(
                    out_ap=gh[:, :, :], in_ap=xtab[half:nt, :],
                    idxs_ap=ihi_sb[:, b * ch * 8:(b + 1) * ch * 8],
                    num_idxs=ch * 128, num_idxs_reg=ch * 128, elem_size=D,
                    single_packet=False)
                nc.gpsimd.dma_gather(
                    out_ap=gad[:, :, :], in_ap=adrep[:, :],
                    idxs_ap=iad_sb[:, b * cb * 8:(b + 1) * cb * 8],
                    num_idxs=cb * 128, num_idxs_reg=cb * 128, elem_size=64,
                    single_packet=False)

                def xg(c):
                    return gl[:, c, :] if c < cl else gh[:, c - cl, :]

                # alpha_src per edge: fused multiply + free-axis reduce
                aspack = bpool.tile([128, cb], F32, tag="aspack")
                for c in range(cb):
                    ttr_t = wpool.tile([128, 128], F32, tag="ttr")
                    nc.vector.scalar_tensor_tensor(
                        out=ttr_t[:, :], in0=xg(c), scalar=1.0,
                        in1=as_sb[:, l, :], op0=ALU.mult, op1=ALU.mult,
                        accum_out=aspack[:, c:c + 1])

                # e = lrelu(as+ad) = .6x + .4|x|; ex = exp(e)
                aepack = bpool.tile([128, cb], F32, tag="aepack")
                nc.vector.tensor_add(aepack[:, :], aspack[:, :], gad[:, :, 0])
                abs_t = bpool.tile([128, cb], F32, tag="abs")
                nc.scalar.activation(abs_t[:, :], aepack[:, :], AF.Abs,
                                     scale=(1.0 - NEG) / 2.0)
                e_t = bpool.tile([128, cb], F32, tag="et")
                nc.vector.scalar_tensor_tensor(
                    out=e_t[:, :], in0=aepack[:, :], scalar=(1.0 + NEG) / 2.0,
                    in1=abs_t[:, :], op0=ALU.mult, op1=ALU.add)
                expack = bpool.tile([128, cb], F32, tag="expack")
                nc.scalar.activation(expack[:, :], e_t[:, :], AF.Exp)

                # A = onehot(dstloc)*ex ; scatter + denominator matmuls
                psum_o = ppool.tile([128, 128], F32, tag="po")
                psum_d = ppool.tile([1, 128], F32, tag="pd")
                for c in range(cb):
                    a_t = wpool.tile([128, 128], F32, tag="a")
                    nc.vector.tensor_scalar(
                        out=a_t[:, :], in0=iota_f[:, :],
                        scalar1=dstloc_sb[:, b * cb + c:b * cb + c + 1],
                        scalar2=expack[:, c:c + 1],
                        op0=ALU.is_equal, op1=ALU.mult)
                    nc.tensor.matmul(out=psum_o[:, :], lhsT=xg(c), rhs=a_t[:, :],
                                     start=(c == 0), stop=(c == cb - 1))
                    nc.tensor.matmul(out=psum_d[:, :], lhsT=ones_sb[:, :],
                                     rhs=a_t[:, :],
                                     start=(c == 0), stop=(c == cb - 1))
                nc.scalar.copy(hacc[:, sl], psum_o[:, :])
                nc.scalar.copy(den_row[:, sl], psum_d[:, :])

            # layer epilogue: out = hacc/(den+eps) + bias (, relu)
            nc.vector.tensor_scalar_add(den_row[:, :], den_row[:, :], EPS)
            nc.vector.reciprocal(den_row[:, :], den_row[:, :])
            for b in range(nblk):
                sl = slice(b * BLK, (b + 1) * BLK)
                rb = wpool.tile([128, BLK], F32, tag="rb")
                nc.gpsimd.partition_broadcast(rb[:, :], den_row[0:1, sl])
                nc.vector.tensor_mul(hacc[:, sl], hacc[:, sl], rb[:, :])
            nc.scalar.activation(
                h_own[:, :], hacc[:, :],
                AF.Relu if l < N_LAYERS - 1 else AF.Identity,
                bias=bias_sb[:, l:l + 1])
            if DEBUG_TAPS:
                nc.sync.dma_start(dbg_h[l, :, :], h_own[:, :])

        nc.sync.dma_start(out_own, h_own[:, :])


def make_nc(cfg):
    nc = bacc.Bacc("TRN2", target_bir_lowering=False, debug=False,
                   num_devices=N_CORES)
    with tile.TileContext(nc) as tc:
        build(nc, tc, cfg)
    nc.compile()
    return nc


def bench_pjrt(nc, in_maps, iters=12):
    """Repeat-execute the compiled module via PJRT and wall-clock it.

    Mirrors bass2jax.run_bass_via_pjrt's multi-core path but without
    donation (kernel fully writes its outputs) so the jitted function can
    be re-invoked on cached device inputs. Returns (results, times_s).
    """
    import time as _time
    import jax
    import jax.numpy as jnp
    from jax.sharding import Mesh, PartitionSpec
    from jax.experimental.shard_map import shard_map
    from concourse import bass2jax as b2j

    b2j.install_neuronx_cc_hook()
    n_cores = len(in_maps)
    partition_name = (nc.partition_id_tensor.name
                      if nc.partition_id_tensor else None)
    in_names, out_names, out_avals, zero_outs = [], [], [], []
    for alloc in nc.m.functions[0].allocations:
        if not isinstance(alloc, mybir.MemoryLocationSet):
            continue
        name = alloc.memorylocations[0].name
        if alloc.kind == "ExternalInput":
            if name != partition_name:
                in_names.append(name)
        elif alloc.kind == "ExternalOutput":
            out_names.append(name)
            shape = tuple(alloc.tensor_shape)
            dtype = mybir.dt.np(alloc.dtype)
            out_avals.append(jax.core.ShapedArray(shape, dtype))
            zero_outs.append(np.zeros(shape, dtype))
    n_params = len(in_names)
    all_in_names = in_names + out_names
    if partition_name is not None:
        all_in_names = all_in_names + [partition_name]

    def _body(*args):
        operands = list(args)
        if partition_name is not None:
            operands.append(b2j.partition_id_tensor())
        outs = b2j._bass_exec_p.bind(
            *operands,
            out_avals=tuple(out_avals),
            in_names=tuple(all_in_names),
            out_names=tuple(out_names),
            lowering_input_output_aliases=(),
            sim_require_finite=True,
            sim_require_nnan=True,
            nc=nc,
        )
        return tuple(outs)

    devices = jax.devices()[:n_cores]
    mesh = Mesh(np.asarray(devices), ("core",))
    n_outs = len(out_names)
    sharded = jax.jit(
        shard_map(_body, mesh=mesh,
                  in_specs=(PartitionSpec("core"),) * (n_params + n_outs),
                  out_specs=(PartitionSpec("core"),) * n_outs,
                  check_rep=False),
        keep_unused=True,
    )
    per_core = [[np.asarray(m[name]) for name in in_names] for m in in_maps]
    concat_in = [np.concatenate([per_core[c][i] for c in range(n_cores)], 0)
                 for i in range(n_params)]
    concat_zeros = [np.zeros((n_cores * z.shape[0], *z.shape[1:]), z.dtype)
                    for z in zero_outs]
    dev_in = [jax.device_put(a) for a in concat_in + concat_zeros]
    out_arrs = sharded(*dev_in)  # warmup + compile
    jax.block_until_ready(out_arrs)
    times = []
    for _ in range(iters):
        t0 = _time.perf_counter()
        o = sharded(*dev_in)
        jax.block_until_ready(o)
        times.append(_time.perf_counter() - t0)
    results = [
        {name: np.asarray(out_arrs[i]).reshape(n_cores, *out_avals[i].shape)[c]
         for i, name in enumerate(out_names)}
        for c in range(n_cores)
    ]
    return results, times


def kernel(z, edge_index, Ws, a_src, a_dst, bias):
    global LAST_EXEC_TIME_NS
    z = np.asarray(z, np.float32)
    edge_index = np.asarray(edge_index)
    Ws = np.asarray(Ws, np.float32)
    a_src = np.asarray(a_src, np.float32)
    a_dst = np.asarray(a_dst, np.float32)
    bias = np.asarray(bias, np.float32)

    cfg = pick_cfg(z.shape[0], edge_index)
    in_maps = prep_host(z, edge_index, Ws, a_src, a_dst, bias, cfg)
    nc = make_nc(cfg)
    global LAST_RESULTS, LAST_RES_OBJ, LAST_BENCH_TIMES
    if BENCH_ITERS:
        results, times = bench_pjrt(nc, in_maps, iters=BENCH_ITERS)
        LAST_BENCH_TIMES = times
        LAST_EXEC_TIME_NS = int(min(times) * 1e9)
        LAST_RESULTS = results
        full = np.concatenate(
            [results[c]["out_own"] for c in range(N_CORES)], axis=1)
        return np.ascontiguousarray(full.T[:cfg.n_nodes]).astype(np.float32)
    sim = MultiCoreSim(nc, num_cores=N_CORES)
    res = sim.run_on_hw_raw(in_maps=in_maps, trace=TRACE)
    LAST_EXEC_TIME_NS = res.exec_time_ns
    LAST_RESULTS = res.results
    LAST_RES_OBJ = res
    full = np.concatenate([res.results[c]["out_own"] for c in range(N_CORES)],
                          axis=1)
    return np.ascontiguousarray(full.T[:cfg.n_nodes]).astype(np.float32)

